# revision 33
# baseline (speedup 1.0000x reference)
"""GRU encoder (embedding lookup + input projection + 512-step GRU) on 8 trn2 NeuronCores.

Strategy (v1): data-parallel over batch. Each of the 8 cores processes
B/8 = 8 sequences end-to-end with no cross-core communication:

  phase 1:  gather embedding rows (indirect DMA), transpose E onto
            partitions via the PE, matmul with the input-projection weight
            (bias folded in through an appended ones-row), stage
            xp = xe @ W_in + b to DRAM scratch.
  phase 2:  512 sequential GRU steps.  The recurrent weight (12 MB fp32)
            stays SBUF-resident; each step streams it through the PE as
            the moving operand (48 matmuls of N=512).  PSUM is pre-seeded
            with xp_t (has_written bits persist after a one-time dummy
            matmul, so start=False accumulation adds h @ R on top), gate
            math runs chunked on DVE/ACT/GPSIMD so it pipelines under the
            PE stream, and h_new is PE-transposed back into the [U, B]
            layout the next step's matmuls need.

Self-contained: hardcodes all shapes; host-side sharding in kernel().
"""

import sys
from contextlib import ExitStack

import numpy as np

try:
    from concourse import bass, mybir, tile
except ImportError:  # pragma: no cover
    sys.path.insert(0, "/opt/trn_rl_repo")
    from concourse import bass, mybir, tile

from concourse.bass_utils import run_bass_kernel_spmd
from concourse.masks import make_identity

B, T, V, E, U = 64, 512, 32000, 300, 1024
G = 3 * U
NCORES = 8
BS = B // NCORES  # 8 sequences per core
P = 128
F32 = mybir.dt.float32
F32R = mybir.dt.float32r  # same bits as fp32; PE streams 1 col/cycle (vs 4 for fp32)
I32 = mybir.dt.int32

AF = mybir.ActivationFunctionType
OP = mybir.AluOpType


class TileCtx(tile.TileContext):
    """TileContext + post-pass: this walrus build encodes at most ONE sync
    wait per instruction, so excess waits are hoisted onto NoOps inserted
    just before the offending instruction (same engine => program order
    guarantees the waits still gate it)."""

    _MAX_WAITS = 1

    def __exit__(self, exc_type, exc_value, traceback):
        ret = super().__exit__(exc_type, exc_value, traceback)
        if exc_type is None:
            self._split_excess_waits()
        return ret

    def _split_excess_waits(self):
        nc = self.nc
        nsplit = 0
        for f in nc.m.functions:
            for bb in f.blocks:
                insts = bb.instructions
                out = []
                for inst in insts:
                    si = getattr(inst, "sync_info", None)
                    waits = list(si.on_wait) if si is not None else []
                    if len(waits) > self._MAX_WAITS:
                        keep = waits[: self._MAX_WAITS]
                        rest = waits[self._MAX_WAITS :]
                        for j in range(0, len(rest), self._MAX_WAITS):
                            nop = mybir.InstNoOp(
                                name=f"waitsplit-{nc.next_id()}", ins=[], outs=[]
                            )
                            nop.engine = inst.engine
                            nop.sync_info = mybir.SyncInfo(
                                on_wait=rest[j : j + self._MAX_WAITS], on_update=[]
                            )
                            out.append(nop)
                            nsplit += 1
                        inst.sync_info = mybir.SyncInfo(
                            on_wait=keep, on_update=list(si.on_update)
                        )
                    out.append(inst)
                if len(out) != len(insts):
                    insts[:] = out


def build_gru_dp(t_steps=T, preseed=True):
    """Build the single-core Bass program (SPMD: all cores run this)."""
    nrows = BS * t_steps  # gather rows, row-major (b, t)
    assert nrows % P == 0
    nchunks = nrows // P
    n_ntiles = G // 512  # 6
    uchunk = 256  # gate-math chunk (h dims per chunk)
    n_uchunks = U // uchunk  # 4

    nc = bass.Bass("TRN2", target_bir_lowering=False, debug=False)

    x_t = nc.dram_tensor("x_tok", [nrows, 1], I32, kind="ExternalInput")
    emb = nc.dram_tensor("emb", [V, E], F32, kind="ExternalInput")
    kmatb = nc.dram_tensor("kmatb", [E, G], F32, kind="ExternalInput")
    # brow = bias0 + [b1_z, b1_r, 0], broadcast to 128 partitions host-side
    brow = nc.dram_tensor("brow", [P, G], F32, kind="ExternalInput")
    rker = nc.dram_tensor("rker", [U, G], F32, kind="ExternalInput")
    b1h = nc.dram_tensor("b1h", [BS, U], F32, kind="ExternalInput")
    hid = nc.dram_tensor("hid", [BS, U], F32, kind="ExternalInput")
    hidt = nc.dram_tensor("hidt", [P, BS * 8], F32, kind="ExternalInput")
    out = nc.dram_tensor("out", [BS, t_steps, U], F32, kind="ExternalOutput")
    xpb = nc.dram_tensor("xpb_scratch", [nrows, G], F32)
    xpb_v = xpb[:].rearrange("(b t) g -> b t g", t=t_steps)

    ET = [128, 128, 44]  # E-tile sizes (300 -> 128+128+44)

    with TileCtx(nc) as tc, ExitStack() as ctx:
        # ---------- persistent pools ----------
        wpool = ctx.enter_context(tc.tile_pool(name="wpool", bufs=1))
        w_sb = []
        for k in range(U // P):
            wk = wpool.tile([P, G], F32R, tag=f"w{k}", name=f"w{k}")
            nc.gpsimd.dma_start(wk[:], rker[k * P : (k + 1) * P, :])
            w_sb.append(wk)
        ident = wpool.tile([P, P], F32, tag="ident")
        make_identity(nc, ident[:])
        b1h_sb = wpool.tile([BS, U], F32, tag="b1h")
        nc.sync.dma_start(b1h_sb[:], b1h[:])
        # persistent state: ping-pong h ([BS, U]) and hT ([P, BS*8])
        h_sb = [wpool.tile([BS, U], F32, tag=f"h{i}", name=f"h{i}") for i in range(2)]
        ht_sb = [wpool.tile([P, BS * 8], F32R, tag=f"ht{i}", name=f"ht{i}") for i in range(2)]
        nc.sync.dma_start(h_sb[0][:], hid[:])
        nc.gpsimd.dma_start(ht_sb[0][:], hidt[:])

        # ---------- phase 1: gather + input projection ----------
        with ExitStack() as p1:
            kpool = p1.enter_context(tc.tile_pool(name="kpool", bufs=1))
            km_sb = []
            r0 = 0
            for k, esz in enumerate(ET):
                kk = kpool.tile([esz, G], F32R, tag=f"km{k}", name=f"km{k}")
                nc.gpsimd.dma_start(kk[:], kmatb[r0 : r0 + esz, :])
                km_sb.append(kk)
                r0 += esz
            brow_sb = kpool.tile([P, G], F32, tag="brow")
            nc.sync.dma_start(brow_sb[:], brow[:])
            gpool = p1.enter_context(tc.tile_pool(name="gpool", bufs=2))
            xppool = p1.enter_context(tc.tile_pool(name="xppool", bufs=2))
            ps1 = p1.enter_context(tc.tile_pool(name="ps1", bufs=2, space="PSUM"))
            tr1 = p1.enter_context(tc.tile_pool(name="tr1", bufs=2, space="PSUM"))

            # chunk order: t-major so early timesteps finish first
            tblocks = max(1, t_steps // P)
            rowblocks = []
            if t_steps >= P:
                for tb in range(tblocks):
                    for b in range(BS):
                        rowblocks.append(b * t_steps + tb * P)
            else:
                rowblocks = [i * P for i in range(nchunks)]

            for r0 in rowblocks:
                idx = gpool.tile([P, 1], I32, tag="idx")
                nc.sync.dma_start(idx[:], x_t[r0 : r0 + P, :])
                xe = gpool.tile([P, E], F32, tag="xe")
                nc.gpsimd.indirect_dma_start(
                    out=xe[:],
                    out_offset=None,
                    in_=emb[:],
                    in_offset=bass.IndirectOffsetOnAxis(ap=idx[:, :1], axis=0),
                )
                # transpose the three E-pieces: xeT_k = xe[:, piece].T
                xet = []
                e0 = 0
                for k, esz in enumerate(ET):
                    w = min(esz, E - e0)  # 128,128,44 valid cols
                    tp = tr1.tile([P, P], F32, tag="tp")
                    nc.tensor.transpose(tp[:w, :], xe[:, e0 : e0 + w], ident[:])
                    xt = gpool.tile([esz, P], F32R, tag=f"xet{k}", name=f"xet{k}")
                    nc.vector.tensor_copy(xt[:w, :], tp[:w, :])
                    xet.append(xt)
                    e0 += w
                xp_sb = xppool.tile([P, G], F32, tag="xp")
                for n in range(n_ntiles):
                    pst = ps1.tile([P, 512], F32, tag="xp_ps")
                    for k in range(3):
                        nc.tensor.matmul(
                            pst[:],
                            xet[k][:],
                            km_sb[k][:, n * 512 : (n + 1) * 512],
                            start=(k == 0),
                            stop=(k == 2),
                        )
                    nc.vector.tensor_tensor(
                        xp_sb[:, n * 512 : (n + 1) * 512],
                        pst[:],
                        brow_sb[:, n * 512 : (n + 1) * 512],
                        op=OP.add,
                    )
                nc.sync.dma_start(xpb[r0 : r0 + P, :], xp_sb[:])

        # ---------- phase 2: recurrence ----------
        with ExitStack() as p2:
            ps2 = p2.enter_context(tc.tile_pool(name="ps2", bufs=1, space="PSUM"))
            tr2 = p2.enter_context(tc.tile_pool(name="tr2", bufs=2, space="PSUM"))
            spool = p2.enter_context(tc.tile_pool(name="spool", bufs=3))
            gtile = p2.enter_context(tc.tile_pool(name="gtile", bufs=3))

            # one PSUM tile per bank so RAW/WAR deps stay bank-granular
            zr_b = [
                ps2.tile([BS, 512], F32, tag=f"zrps{c}", name=f"zrps{c}")
                for c in range(4)
            ]
            h_b = [
                ps2.tile([BS, 512], F32, tag=f"hps{n}", name=f"hps{n}")
                for n in range(2)
            ]

            stage = wpool.tile([1, 512], F32, tag="stage")
            ones1 = wpool.tile([1, BS], F32R, tag="ones1")
            nc.vector.memset(stage[:], 1.0)
            nc.vector.tensor_copy(ones1[:], stage[:, :BS])
            b1h_row_sb = wpool.tile([1, U], F32R, tag="b1hrow")
            nc.vector.tensor_copy(b1h_row_sb[:], b1h_sb[0:1, :])
            if preseed:
                # one-time dummy matmuls: set has_written on the zr banks
                zlhs = wpool.tile([1, BS], F32R, tag="zlhs")
                zrhs = wpool.tile([1, 512], F32R, tag="zrhs")
                nc.vector.memset(stage[:], 0.0)
                nc.vector.tensor_copy(zlhs[:], stage[:, :BS])
                nc.vector.tensor_copy(zrhs[:], stage[:])
                for pt in zr_b + h_b:
                    nc.tensor.matmul(pt[:], zlhs[:], zrhs[:], start=True, stop=True)

            for t in range(t_steps):
                h_cur, h_nxt = h_sb[t % 2], h_sb[(t + 1) % 2]
                ht_cur, ht_nxt = ht_sb[t % 2], ht_sb[(t + 1) % 2]

                xp_t = spool.tile([BS, G], F32, tag="xp_t")
                nc.sync.dma_start(xp_t[:], xpb_v[:, t, :])

                if preseed:
                    # h pre-seeds first: the h banks lead the matmul stream
                    for n in range(2):
                        nc.scalar.copy(h_b[n][:], b1h_sb[:, n * 512 : (n + 1) * 512])
                    for c in range(4):
                        nc.scalar.copy(zr_b[c][:], xp_t[:, c * 512 : (c + 1) * 512])
                # recurrent matmuls: psum += h @ R (W streams as rhs).
                # h-gate banks first, then the per-chunk interleaved z|r banks,
                # so chunk-c gate math starts right after its own bank.
                # h banks first (start=True clears them — no ACT pre-seed
                # on the h path; bias1_h lands via a K=1 ones-row matmul),
                # then the per-chunk z|r banks whose pre-seed carries xp_t.
                for n in range(2):
                    for k in range(8):
                        nc.tensor.matmul(
                            h_b[n][:],
                            ht_cur[:, 8 * k : 8 * k + 8],
                            w_sb[k][:, 2048 + n * 512 : 2048 + (n + 1) * 512],
                            start=(not preseed and k == 0),
                            stop=(k == 7),
                            skip_group_check=preseed,
                        )
                for n in range(4):
                    for k in range(8):
                        nc.tensor.matmul(
                            zr_b[n][:],
                            ht_cur[:, 8 * k : 8 * k + 8],
                            w_sb[k][:, n * 512 : (n + 1) * 512],
                            start=(not preseed and k == 0),
                            stop=(k == 7),
                            skip_group_check=preseed,
                        )

                for c in range(n_uchunks):
                    cs = slice(c * uchunk, (c + 1) * uchunk)
                    zr = gtile.tile([BS, 2 * uchunk], F32, tag="zr")
                    # z|r for chunk c live contiguously in psum bank c
                    nc.scalar.activation(zr[:], zr_b[c][:], AF.Sigmoid)
                    z = zr[:, :uchunk]
                    r = zr[:, uchunk:]
                    u1 = gtile.tile([BS, uchunk], F32, tag="u1")
                    # u1 = 1 - z   (gpsimd, off the DVE)
                    nc.gpsimd.tensor_scalar(u1[:], z, -1.0, 1.0, OP.mult, OP.add)
                    rrh = gtile.tile([BS, uchunk], F32, tag="rrh")
                    nc.vector.tensor_tensor(
                        rrh[:], r,
                        h_b[c // 2][:, (c % 2) * 256 : (c % 2) * 256 + 256],
                        op=OP.mult,
                    )
                    nc.vector.tensor_tensor(
                        rrh[:], rrh[:], xp_t[:, 2 * U + c * uchunk : 2 * U + (c + 1) * uchunk],
                        op=OP.add,
                    )
                    hh = gtile.tile([BS, uchunk], F32, tag="hh")
                    nc.scalar.activation(hh[:], rrh[:], AF.Tanh)
                    a = gtile.tile([BS, uchunk], F32, tag="a")
                    nc.gpsimd.tensor_tensor(a[:], z, h_cur[:, cs], op=OP.mult)
                    nc.vector.tensor_tensor(hh[:], u1[:], hh[:], op=OP.mult)
                    nc.vector.tensor_tensor(h_nxt[:, cs], a[:], hh[:], op=OP.add)
                    # transpose the two 128-wide halves back into hT layout
                    for i in range(2):
                        d0 = c * uchunk + i * P
                        tp = tr2.tile([P, BS], F32, tag="tp2")
                        nc.tensor.transpose(
                            tp[:], h_nxt[:, d0 : d0 + P], ident[:BS, :BS]
                        )
                        nc.vector.tensor_copy(
                            ht_nxt[:, (d0 // P) * BS : (d0 // P) * BS + BS], tp[:]
                        )
                nc.sync.dma_start(out[:, t, :], h_nxt[:])

    return nc


_CACHE = {}


def _get_nc():
    if "nc" not in _CACHE:
        _CACHE["nc"] = build_gru_dp()
    return _CACHE["nc"]


def make_in_maps(x, hidden, embedding_matrix, kernel, recurrent_kernel, bias,
                 t_steps=T):
    """Host-side sharding: per-core input dicts."""
    x = np.asarray(x).astype(np.int32)
    hidden = np.asarray(hidden, dtype=np.float32)
    emb = np.ascontiguousarray(np.asarray(embedding_matrix, dtype=np.float32))
    kmat = np.asarray(kernel, dtype=np.float32)
    rker = np.ascontiguousarray(np.asarray(recurrent_kernel, dtype=np.float32))
    bias = np.asarray(bias, dtype=np.float32)

    brow = bias[0].copy()
    brow[: 2 * U] += bias[1][: 2 * U]  # bias1 for z,r folded into xp
    # permute gate columns: zr region becomes per-chunk [z_c(256) | r_c(256)]
    perm = np.empty(G, np.int64)
    uc = 256
    for c in range(U // uc):
        perm[512 * c : 512 * c + uc] = np.arange(c * uc, (c + 1) * uc)
        perm[512 * c + uc : 512 * (c + 1)] = U + np.arange(c * uc, (c + 1) * uc)
    perm[2 * U :] = np.arange(2 * U, G)
    kmat = kmat[:, perm]
    rker = rker[:, perm]
    brow = brow[perm]
    brow_bc = np.ascontiguousarray(np.broadcast_to(brow, (P, G))).astype(np.float32)
    b1h_row = bias[1][2 * U :]

    in_maps = []
    for c in range(NCORES):
        xs = x[c * BS : (c + 1) * BS, :t_steps]
        hs = hidden[c * BS : (c + 1) * BS]
        # hT packed: [128, BS*8]; col 8k+j = h[j, 128k:128(k+1)]
        hidt = np.ascontiguousarray(
            hs.T.reshape(U // P, P, BS).transpose(1, 0, 2).reshape(P, BS * (U // P))
        )
        in_maps.append(
            {
                "x_tok": np.ascontiguousarray(xs.reshape(-1, 1)),
                "emb": emb,
                "kmatb": np.ascontiguousarray(kmat),
                "brow": brow_bc,
                "rker": rker,
                "b1h": np.ascontiguousarray(
                    np.broadcast_to(b1h_row, (BS, U))
                ).astype(np.float32),
                "hid": np.ascontiguousarray(hs),
                "hidt": hidt,
            }
        )
    return in_maps


def kernel(x, hidden, embedding_matrix, kernel, recurrent_kernel, bias,
           _trace=False):
    nc = _get_nc()
    in_maps = make_in_maps(
        x, hidden, embedding_matrix, kernel, recurrent_kernel, bias
    )
    res = run_bass_kernel_spmd(
        nc, in_maps, core_ids=list(range(NCORES)), trace=_trace
    )
    outs = np.concatenate([res.results[c]["out"] for c in range(NCORES)], axis=0)
    state = np.ascontiguousarray(outs[:, -1, :])
    if _trace:
        kernel.last_results = res
    return outs, state


# revision 35
# speedup vs baseline: 1.0177x; 1.0177x over previous
"""GRU encoder (embedding lookup + input projection + 512-step GRU) on 8 trn2 NeuronCores.

Strategy (v1): data-parallel over batch. Each of the 8 cores processes
B/8 = 8 sequences end-to-end with no cross-core communication:

  phase 1:  gather embedding rows (indirect DMA), transpose E onto
            partitions via the PE, matmul with the input-projection weight
            (bias folded in through an appended ones-row), stage
            xp = xe @ W_in + b to DRAM scratch.
  phase 2:  512 sequential GRU steps.  The recurrent weight (12 MB fp32)
            stays SBUF-resident; each step streams it through the PE as
            the moving operand (48 matmuls of N=512).  PSUM is pre-seeded
            with xp_t (has_written bits persist after a one-time dummy
            matmul, so start=False accumulation adds h @ R on top), gate
            math runs chunked on DVE/ACT/GPSIMD so it pipelines under the
            PE stream, and h_new is PE-transposed back into the [U, B]
            layout the next step's matmuls need.

Self-contained: hardcodes all shapes; host-side sharding in kernel().
"""

import sys
from contextlib import ExitStack

import numpy as np

try:
    from concourse import bass, mybir, tile
except ImportError:  # pragma: no cover
    sys.path.insert(0, "/opt/trn_rl_repo")
    from concourse import bass, mybir, tile

from concourse.bass_utils import run_bass_kernel_spmd
from concourse.masks import make_identity

B, T, V, E, U = 64, 512, 32000, 300, 1024
G = 3 * U
NCORES = 8
BS = B // NCORES  # 8 sequences per core
P = 128
F32 = mybir.dt.float32
F32R = mybir.dt.float32r  # same bits as fp32; PE streams 1 col/cycle (vs 4 for fp32)
I32 = mybir.dt.int32

AF = mybir.ActivationFunctionType
OP = mybir.AluOpType


class TileCtx(tile.TileContext):
    """TileContext + post-pass: this walrus build encodes at most ONE sync
    wait per instruction, so excess waits are hoisted onto NoOps inserted
    just before the offending instruction (same engine => program order
    guarantees the waits still gate it)."""

    _MAX_WAITS = 1

    def __exit__(self, exc_type, exc_value, traceback):
        ret = super().__exit__(exc_type, exc_value, traceback)
        if exc_type is None:
            self._split_excess_waits()
        return ret

    def _split_excess_waits(self):
        nc = self.nc
        nsplit = 0
        for f in nc.m.functions:
            for bb in f.blocks:
                insts = bb.instructions
                out = []
                for inst in insts:
                    si = getattr(inst, "sync_info", None)
                    waits = list(si.on_wait) if si is not None else []
                    if len(waits) > self._MAX_WAITS:
                        keep = waits[: self._MAX_WAITS]
                        rest = waits[self._MAX_WAITS :]
                        for j in range(0, len(rest), self._MAX_WAITS):
                            nop = mybir.InstNoOp(
                                name=f"waitsplit-{nc.next_id()}", ins=[], outs=[]
                            )
                            nop.engine = inst.engine
                            nop.sync_info = mybir.SyncInfo(
                                on_wait=rest[j : j + self._MAX_WAITS], on_update=[]
                            )
                            out.append(nop)
                            nsplit += 1
                        inst.sync_info = mybir.SyncInfo(
                            on_wait=keep, on_update=list(si.on_update)
                        )
                    out.append(inst)
                if len(out) != len(insts):
                    insts[:] = out


def build_gru_dp(t_steps=T, preseed=True):
    """Build the single-core Bass program (SPMD: all cores run this)."""
    nrows = BS * t_steps  # gather rows, row-major (b, t)
    assert nrows % P == 0
    nchunks = nrows // P
    n_ntiles = G // 512  # 6
    uchunk = 256  # gate-math chunk (h dims per chunk)
    n_uchunks = U // uchunk  # 4

    nc = bass.Bass("TRN2", target_bir_lowering=False, debug=False)

    x_t = nc.dram_tensor("x_tok", [nrows, 1], I32, kind="ExternalInput")
    emb = nc.dram_tensor("emb", [V, E], F32, kind="ExternalInput")
    kmatb = nc.dram_tensor("kmatb", [E, G], F32, kind="ExternalInput")
    # brow = bias0 + [b1_z, b1_r, 0], broadcast to 128 partitions host-side
    brow = nc.dram_tensor("brow", [P, G], F32, kind="ExternalInput")
    rker = nc.dram_tensor("rker", [U, G], F32, kind="ExternalInput")
    b1h = nc.dram_tensor("b1h", [BS, U], F32, kind="ExternalInput")
    hid = nc.dram_tensor("hid", [BS, U], F32, kind="ExternalInput")
    hidt = nc.dram_tensor("hidt", [P, BS * 8], F32, kind="ExternalInput")
    out = nc.dram_tensor("out", [BS, t_steps, U], F32, kind="ExternalOutput")
    xpb = nc.dram_tensor("xpb_scratch", [nrows, G], F32)
    xpb_v = xpb[:].rearrange("(b t) g -> b t g", t=t_steps)

    ET = [128, 128, 44]  # E-tile sizes (300 -> 128+128+44)

    with TileCtx(nc) as tc, ExitStack() as ctx:
        # ---------- persistent pools ----------
        wpool = ctx.enter_context(tc.tile_pool(name="wpool", bufs=1))
        w_sb = []
        for k in range(U // P):
            wk = wpool.tile([P, G], F32R, tag=f"w{k}", name=f"w{k}")
            nc.gpsimd.dma_start(wk[:], rker[k * P : (k + 1) * P, :])
            w_sb.append(wk)
        ident = wpool.tile([P, P], F32, tag="ident")
        make_identity(nc, ident[:])
        b1h_sb = wpool.tile([BS, U], F32, tag="b1h")
        nc.sync.dma_start(b1h_sb[:], b1h[:])
        # persistent state: ping-pong h ([BS, U]) and hT ([P, BS*8])
        h_sb = [wpool.tile([BS, U], F32, tag=f"h{i}", name=f"h{i}") for i in range(2)]
        ht_sb = [wpool.tile([P, BS * 8], F32R, tag=f"ht{i}", name=f"ht{i}") for i in range(2)]
        nc.sync.dma_start(h_sb[0][:], hid[:])
        nc.gpsimd.dma_start(ht_sb[0][:], hidt[:])

        # ---------- phase 1: gather + input projection ----------
        with ExitStack() as p1:
            kpool = p1.enter_context(tc.tile_pool(name="kpool", bufs=1))
            km_sb = []
            r0 = 0
            for k, esz in enumerate(ET):
                kk = kpool.tile([esz, G], F32R, tag=f"km{k}", name=f"km{k}")
                nc.gpsimd.dma_start(kk[:], kmatb[r0 : r0 + esz, :])
                km_sb.append(kk)
                r0 += esz
            brow_sb = kpool.tile([P, G], F32, tag="brow")
            nc.sync.dma_start(brow_sb[:], brow[:])
            gpool = p1.enter_context(tc.tile_pool(name="gpool", bufs=2))
            xppool = p1.enter_context(tc.tile_pool(name="xppool", bufs=2))
            ps1 = p1.enter_context(tc.tile_pool(name="ps1", bufs=2, space="PSUM"))
            tr1 = p1.enter_context(tc.tile_pool(name="tr1", bufs=2, space="PSUM"))

            # chunk order: t-major so early timesteps finish first
            tblocks = max(1, t_steps // P)
            rowblocks = []
            if t_steps >= P:
                for tb in range(tblocks):
                    for b in range(BS):
                        rowblocks.append(b * t_steps + tb * P)
            else:
                rowblocks = [i * P for i in range(nchunks)]

            for r0 in rowblocks:
                idx = gpool.tile([P, 1], I32, tag="idx")
                nc.sync.dma_start(idx[:], x_t[r0 : r0 + P, :])
                xe = gpool.tile([P, E], F32, tag="xe")
                nc.gpsimd.indirect_dma_start(
                    out=xe[:],
                    out_offset=None,
                    in_=emb[:],
                    in_offset=bass.IndirectOffsetOnAxis(ap=idx[:, :1], axis=0),
                )
                # transpose the three E-pieces: xeT_k = xe[:, piece].T
                xet = []
                e0 = 0
                for k, esz in enumerate(ET):
                    w = min(esz, E - e0)  # 128,128,44 valid cols
                    tp = tr1.tile([P, P], F32, tag="tp")
                    nc.tensor.transpose(tp[:w, :], xe[:, e0 : e0 + w], ident[:])
                    xt = gpool.tile([esz, P], F32R, tag=f"xet{k}", name=f"xet{k}")
                    nc.vector.tensor_copy(xt[:w, :], tp[:w, :])
                    xet.append(xt)
                    e0 += w
                xp_sb = xppool.tile([P, G], F32, tag="xp")
                for n in range(n_ntiles):
                    pst = ps1.tile([P, 512], F32, tag="xp_ps")
                    for k in range(3):
                        nc.tensor.matmul(
                            pst[:],
                            xet[k][:],
                            km_sb[k][:, n * 512 : (n + 1) * 512],
                            start=(k == 0),
                            stop=(k == 2),
                        )
                    nc.vector.tensor_tensor(
                        xp_sb[:, n * 512 : (n + 1) * 512],
                        pst[:],
                        brow_sb[:, n * 512 : (n + 1) * 512],
                        op=OP.add,
                    )
                nc.sync.dma_start(xpb[r0 : r0 + P, :], xp_sb[:])

        # ---------- phase 2: recurrence ----------
        with ExitStack() as p2:
            ps2 = p2.enter_context(tc.tile_pool(name="ps2", bufs=1, space="PSUM"))
            tr2 = p2.enter_context(tc.tile_pool(name="tr2", bufs=2, space="PSUM"))
            spool = p2.enter_context(tc.tile_pool(name="spool", bufs=3))
            gtile = p2.enter_context(tc.tile_pool(name="gtile", bufs=3))

            # one PSUM tile per bank so RAW/WAR deps stay bank-granular
            zr_b = [
                ps2.tile([BS, 512], F32, tag=f"zrps{c}", name=f"zrps{c}")
                for c in range(4)
            ]
            h_b = [
                ps2.tile([BS, 512], F32, tag=f"hps{n}", name=f"hps{n}")
                for n in range(2)
            ]

            stage = wpool.tile([1, 512], F32, tag="stage")
            ones1 = wpool.tile([1, BS], F32R, tag="ones1")
            nc.vector.memset(stage[:], 1.0)
            nc.vector.tensor_copy(ones1[:], stage[:, :BS])
            b1h_row_sb = wpool.tile([1, U], F32R, tag="b1hrow")
            nc.vector.tensor_copy(b1h_row_sb[:], b1h_sb[0:1, :])
            if preseed:
                # one-time dummy matmuls: set has_written on the zr banks
                zlhs = wpool.tile([1, BS], F32R, tag="zlhs")
                zrhs = wpool.tile([1, 512], F32R, tag="zrhs")
                nc.vector.memset(stage[:], 0.0)
                nc.vector.tensor_copy(zlhs[:], stage[:, :BS])
                nc.vector.tensor_copy(zrhs[:], stage[:])
                for pt in zr_b + h_b:
                    nc.tensor.matmul(pt[:], zlhs[:], zrhs[:], start=True, stop=True)

            for t in range(t_steps):
                h_cur, h_nxt = h_sb[t % 2], h_sb[(t + 1) % 2]
                ht_cur, ht_nxt = ht_sb[t % 2], ht_sb[(t + 1) % 2]

                xp_t = spool.tile([BS, G], F32, tag="xp_t")
                nc.sync.dma_start(xp_t[:], xpb_v[:, t, :])

                if preseed:
                    # h pre-seeds first: the h banks lead the matmul stream
                    for n in range(2):
                        nc.scalar.copy(h_b[n][:], b1h_sb[:, n * 512 : (n + 1) * 512])
                    for c in range(4):
                        nc.scalar.copy(zr_b[c][:], xp_t[:, c * 512 : (c + 1) * 512])
                # recurrent matmuls: psum += h @ R (W streams as rhs).
                # h-gate banks first, then the per-chunk interleaved z|r banks,
                # so chunk-c gate math starts right after its own bank.
                # h banks first (start=True clears them — no ACT pre-seed
                # on the h path; bias1_h lands via a K=1 ones-row matmul),
                # then the per-chunk z|r banks whose pre-seed carries xp_t.
                for n in range(2):
                    for k in range(8):
                        nc.tensor.matmul(
                            h_b[n][:],
                            ht_cur[:, 8 * k : 8 * k + 8],
                            w_sb[k][:, 2048 + n * 512 : 2048 + (n + 1) * 512],
                            start=(not preseed and k == 0),
                            stop=(k == 7),
                            skip_group_check=preseed,
                        )
                for n in range(4):
                    for k in range(8):
                        nc.tensor.matmul(
                            zr_b[n][:],
                            ht_cur[:, 8 * k : 8 * k + 8],
                            w_sb[k][:, n * 512 : (n + 1) * 512],
                            start=(not preseed and k == 0),
                            stop=(k == 7),
                            skip_group_check=preseed,
                        )

                for c in range(n_uchunks):
                    cs = slice(c * uchunk, (c + 1) * uchunk)
                    zr = gtile.tile([BS, 2 * uchunk], F32, tag="zr")
                    # z|r for chunk c live contiguously in psum bank c
                    nc.scalar.activation(zr[:], zr_b[c][:], AF.Sigmoid)
                    z = zr[:, :uchunk]
                    r = zr[:, uchunk:]
                    u1 = gtile.tile([BS, uchunk], F32, tag="u1")
                    # u1 = 1 - z   (gpsimd, off the DVE)
                    nc.gpsimd.tensor_scalar(u1[:], z, -1.0, 1.0, OP.mult, OP.add)
                    rrh = gtile.tile([BS, uchunk], F32, tag="rrh")
                    nc.vector.tensor_tensor(
                        rrh[:], r,
                        h_b[c // 2][:, (c % 2) * 256 : (c % 2) * 256 + 256],
                        op=OP.mult,
                    )
                    nc.vector.tensor_tensor(
                        rrh[:], rrh[:], xp_t[:, 2 * U + c * uchunk : 2 * U + (c + 1) * uchunk],
                        op=OP.add,
                    )
                    hh = gtile.tile([BS, uchunk], F32, tag="hh")
                    nc.scalar.activation(hh[:], rrh[:], AF.Tanh)
                    a = gtile.tile([BS, uchunk], F32, tag="a")
                    nc.gpsimd.tensor_tensor(a[:], z, h_cur[:, cs], op=OP.mult)
                    nc.vector.tensor_tensor(hh[:], u1[:], hh[:], op=OP.mult)
                    nc.vector.tensor_tensor(h_nxt[:, cs], a[:], hh[:], op=OP.add)
                    # transpose the two 128-wide halves back into hT layout
                    for i in range(2):
                        d0 = c * uchunk + i * P
                        tp = tr2.tile([P, BS], F32, tag="tp2")
                        nc.tensor.transpose(
                            tp[:], h_nxt[:, d0 : d0 + P], ident[:BS, :BS]
                        )
                        nc.vector.tensor_copy(
                            ht_nxt[:, (d0 // P) * BS : (d0 // P) * BS + BS], tp[:]
                        )
                nc.sync.dma_start(out[:, t, :], h_nxt[:])

    return nc


_CACHE = {}


def _get_nc():
    if "nc" not in _CACHE:
        _CACHE["nc"] = build_gru_dp()
    return _CACHE["nc"]


def make_in_maps(x, hidden, embedding_matrix, kernel, recurrent_kernel, bias,
                 t_steps=T):
    """Host-side sharding: per-core input dicts."""
    x = np.asarray(x).astype(np.int32)
    hidden = np.asarray(hidden, dtype=np.float32)
    emb = np.ascontiguousarray(np.asarray(embedding_matrix, dtype=np.float32))
    kmat = np.asarray(kernel, dtype=np.float32)
    rker = np.ascontiguousarray(np.asarray(recurrent_kernel, dtype=np.float32))
    bias = np.asarray(bias, dtype=np.float32)

    brow = bias[0].copy()
    brow[: 2 * U] += bias[1][: 2 * U]  # bias1 for z,r folded into xp
    # permute gate columns: zr region becomes per-chunk [z_c(256) | r_c(256)]
    perm = np.empty(G, np.int64)
    uc = 256
    for c in range(U // uc):
        perm[512 * c : 512 * c + uc] = np.arange(c * uc, (c + 1) * uc)
        perm[512 * c + uc : 512 * (c + 1)] = U + np.arange(c * uc, (c + 1) * uc)
    perm[2 * U :] = np.arange(2 * U, G)
    kmat = kmat[:, perm]
    rker = rker[:, perm]
    brow = brow[perm]
    brow_bc = np.ascontiguousarray(np.broadcast_to(brow, (P, G))).astype(np.float32)
    b1h_row = bias[1][2 * U :]

    in_maps = []
    for c in range(NCORES):
        xs = x[c * BS : (c + 1) * BS, :t_steps]
        hs = hidden[c * BS : (c + 1) * BS]
        # hT packed: [128, BS*8]; col 8k+j = h[j, 128k:128(k+1)]
        hidt = np.ascontiguousarray(
            hs.T.reshape(U // P, P, BS).transpose(1, 0, 2).reshape(P, BS * (U // P))
        )
        in_maps.append(
            {
                "x_tok": np.ascontiguousarray(xs.reshape(-1, 1)),
                "emb": emb,
                "kmatb": np.ascontiguousarray(kmat),
                "brow": brow_bc,
                "rker": rker,
                "b1h": np.ascontiguousarray(
                    np.broadcast_to(b1h_row, (BS, U))
                ).astype(np.float32),
                "hid": np.ascontiguousarray(hs),
                "hidt": hidt,
            }
        )
    return in_maps


def kernel(x, hidden, embedding_matrix, kernel, recurrent_kernel, bias,
           _trace=False):
    nc = _get_nc()
    in_maps = make_in_maps(
        x, hidden, embedding_matrix, kernel, recurrent_kernel, bias
    )
    res = run_bass_kernel_spmd(
        nc, in_maps, core_ids=list(range(NCORES)), trace=_trace
    )
    outs = np.concatenate([res.results[c]["out"] for c in range(NCORES)], axis=0)
    state = np.ascontiguousarray(outs[:, -1, :])
    if _trace:
        kernel.last_results = res
    return outs, state
